# revision 1
# baseline (speedup 1.0000x reference)
"""Cross-attention (single-head, residual) Bass/Tile kernel for Trainium2.

Problem: y = x + (softmax((x' Wq + bq)(ctx Wk + bk)^T / sqrt(C)) (ctx Wv + bv)) Wo + bo
  x: [B=8, C=512, H=64, W=64], context: [B=8, Lc=512, CTX=768]

Sharding: pure data-parallel over batch — one batch element per NeuronCore,
no collectives.  All matmuls run in bf16 with fp32 PSUM accumulation; the
residual add and final output stay fp32.  Layout trick: every matmul is
arranged so both operands are contraction-major in SBUF (out = lhsT.T @ rhs),
which makes x ([C, HW]) and all weight matrices usable in their natural
layouts; only `context` needs an on-chip transpose (PE transpose, 24 tiles).

Softmax over the key axis is computed in the [lc, hw] ("transposed") layout:
no max-subtraction is needed (logits are O(1) by construction), the
denominator is a ones-vector matmul partition reduction, and the division is
folded into the PSUM eviction of attn@V via a K=1 broadcast matmul.
"""

import numpy as np

B = 8
C = 512
CTX = 768
Lc = 512
HH = 64
WW = 64
HW = HH * WW          # 4096
N_CORES = 8
P = 128
HT = 512              # hw tile (free-dim) width
N_HT = HW // HT       # 8
KC = C // P           # 4
KX = CTX // P         # 6
KL = Lc // P          # 4
SCALE = float(C) ** -0.5

_cache = {}


def _build_nc():
    import concourse.mybir as mybir
    import concourse.bass as bass
    import concourse.tile as tile
    from concourse import bacc
    from concourse.masks import make_identity

    f32 = mybir.dt.float32
    bf16 = mybir.dt.bfloat16
    AF = mybir.ActivationFunctionType

    nc = bacc.Bacc("TRN2", target_bir_lowering=False, debug=False,
                   num_devices=N_CORES)

    x_d = nc.dram_tensor("x", [C, HW], f32, kind="ExternalInput").ap()
    ctx_d = nc.dram_tensor("ctx", [Lc, CTX], f32, kind="ExternalInput").ap()
    wq_d = nc.dram_tensor("wq", [C, C], f32, kind="ExternalInput").ap()
    wk_d = nc.dram_tensor("wk", [CTX, C], f32, kind="ExternalInput").ap()
    wv_d = nc.dram_tensor("wv", [CTX, C], f32, kind="ExternalInput").ap()
    wo_d = nc.dram_tensor("wo", [C, C], f32, kind="ExternalInput").ap()
    bq_d = nc.dram_tensor("bq", [C], f32, kind="ExternalInput").ap()
    bk_d = nc.dram_tensor("bk", [C], f32, kind="ExternalInput").ap()
    bv_d = nc.dram_tensor("bv", [C], f32, kind="ExternalInput").ap()
    bo_d = nc.dram_tensor("bo", [C], f32, kind="ExternalInput").ap()
    y_d = nc.dram_tensor("y", [C, HW], f32, kind="ExternalOutput").ap()

    x_r = x_d.rearrange("(ko p) hw -> p ko hw", p=P)      # [128, 4, 4096]
    y_r = y_d.rearrange("(ko p) hw -> p ko hw", p=P)
    ctx_r = ctx_d.rearrange("(lo p) cx -> p lo cx", p=P)  # [128, 4, 768]
    wq_r = wq_d.rearrange("(ko p) c -> p ko c", p=P)      # [128, 4, 512]
    wk_r = wk_d.rearrange("(ko p) c -> p ko c", p=P)      # [128, 6, 512]
    wv_r = wv_d.rearrange("(ko p) c -> p ko c", p=P)
    wo_r = wo_d.rearrange("(ko p) c -> p ko c", p=P)

    with tile.TileContext(nc) as tc:
        with (
            tc.tile_pool(name="const", bufs=1) as const,
            tc.tile_pool(name="stage", bufs=1) as stage,
            tc.tile_pool(name="xin", bufs=2) as xin,
            tc.tile_pool(name="work", bufs=2) as work,
            tc.tile_pool(name="yout", bufs=2) as yout,
            tc.tile_pool(name="small", bufs=3) as small,
            tc.tile_pool(name="psum", bufs=4, space="PSUM") as psum,
            tc.tile_pool(name="psum_s", bufs=2, space="PSUM") as psum_s,
            tc.tile_pool(name="psum_bc", bufs=2, space="PSUM") as psum_bc,
        ):
            # ---------------- phase A: weights / context prep ----------------
            ident_f = const.tile([P, P], f32, name="ident_f", tag="ident")
            make_identity(nc, ident_f)
            ones_col = const.tile([P, 1], bf16, name="ones_col", tag="ones_c")
            nc.vector.memset(ones_col, 1.0)
            ones_row = const.tile([1, P], bf16, name="ones_row", tag="ones_r")
            nc.vector.memset(ones_row, 1.0)

            # biases: per-partition layout [128, KC]
            bq_t = const.tile([P, KC], f32, name="bq_t", tag="bq")
            bk_t = const.tile([P, KC], f32, name="bk_t", tag="bk")
            bo_t = const.tile([P, KC], f32, name="bo_t", tag="bo")
            with nc.allow_non_contiguous_dma(reason="tiny one-time bias loads"):
                nc.sync.dma_start(out=bq_t, in_=bq_d.rearrange("(ko p) -> p ko", p=P))
                nc.sync.dma_start(out=bk_t, in_=bk_d.rearrange("(ko p) -> p ko", p=P))
                nc.sync.dma_start(out=bo_t, in_=bo_d.rearrange("(ko p) -> p ko", p=P))
            # bv broadcast across partitions: [128, 512]
            bv_bc = const.tile([P, C], f32, name="bv_bc", tag="bv")
            bv_src = bass.AP(tensor=bv_d.tensor, offset=bv_d.offset,
                             ap=[[0, P]] + list(bv_d.ap))
            nc.sync.dma_start(out=bv_bc, in_=bv_src)

            # weights fp32 staging -> bf16
            wq_f = stage.tile([P, KC, C], f32, name="wq_f", tag="wq_f")
            wk_f = stage.tile([P, KX, C], f32, name="wk_f", tag="wk_f")
            wv_f = stage.tile([P, KX, C], f32, name="wv_f", tag="wv_f")
            wo_f = stage.tile([P, KC, C], f32, name="wo_f", tag="wo_f")
            ctx_f = stage.tile([P, KL, CTX], f32, name="ctx_f", tag="ctx_f")
            nc.sync.dma_start(out=wq_f, in_=wq_r)
            nc.sync.dma_start(out=wk_f, in_=wk_r)
            nc.sync.dma_start(out=wv_f, in_=wv_r)
            nc.sync.dma_start(out=wo_f, in_=wo_r)
            nc.sync.dma_start(out=ctx_f, in_=ctx_r)

            wq_b = const.tile([P, KC, C], bf16, name="wq_b", tag="wq_b")
            wk_b = const.tile([P, KX, C], bf16, name="wk_b", tag="wk_b")
            wv_b = const.tile([P, KX, C], bf16, name="wv_b", tag="wv_b")
            wo_b = const.tile([P, KC, C], bf16, name="wo_b", tag="wo_b")
            nc.gpsimd.tensor_copy(out=wq_b, in_=wq_f)
            nc.gpsimd.tensor_copy(out=wk_b, in_=wk_f)
            nc.gpsimd.tensor_copy(out=wv_b, in_=wv_f)
            nc.gpsimd.tensor_copy(out=wo_b, in_=wo_f)

            # context transpose: ctxT [128(cx), KX, Lc] bf16
            ctxT_b = const.tile([P, KX, Lc], bf16, name="ctxT_b", tag="ctxT")
            for lo in range(KL):
                for cx in range(KX):
                    ps_t = psum.tile([P, P], f32, tag="mm", name=f"ps_t_{lo}_{cx}")
                    nc.tensor.transpose(ps_t, ctx_f[:, lo, cx * P:(cx + 1) * P],
                                        ident_f)
                    nc.vector.tensor_copy(out=ctxT_b[:, cx, lo * P:(lo + 1) * P],
                                          in_=ps_t)

            # kT [128(c), KC, Lc] = (ctx Wk + bk)^T ; v [128(lc), KL, C] = ctx Wv + bv
            kT_b = const.tile([P, KC, Lc], bf16, name="kT_b", tag="kT")
            v_b = const.tile([P, KL, C], bf16, name="v_b", tag="v")
            for mc in range(KC):
                ps = psum.tile([P, Lc], f32, tag="mm", name=f"ps_k_{mc}")
                for cx in range(KX):
                    nc.tensor.matmul(ps, wk_b[:, cx, mc * P:(mc + 1) * P],
                                     ctxT_b[:, cx, :],
                                     start=(cx == 0), stop=(cx == KX - 1))
                nc.scalar.activation(kT_b[:, mc, :], ps, AF.Identity,
                                     bias=bk_t[:, mc:mc + 1])
            for ml in range(KL):
                ps = psum.tile([P, C], f32, tag="mm", name=f"ps_v_{ml}")
                for cx in range(KX):
                    nc.tensor.matmul(ps, ctxT_b[:, cx, ml * P:(ml + 1) * P],
                                     wv_b[:, cx, :],
                                     start=(cx == 0), stop=(cx == KX - 1))
                nc.vector.tensor_add(out=v_b[:, ml, :], in0=ps, in1=bv_bc)

            # ---------------- phase B: stream over hw tiles ----------------
            for h in range(N_HT):
                hs = slice(h * HT, (h + 1) * HT)

                x_f = xin.tile([P, KC, HT], f32, tag="x_f", name=f"x_f_{h}")
                nc.sync.dma_start(out=x_f, in_=x_r[:, :, hs])
                x_b = work.tile([P, KC, HT], bf16, tag="x_b", name=f"x_b_{h}")
                nc.gpsimd.tensor_copy(out=x_b, in_=x_f)

                # qT [c, hw]
                qT = work.tile([P, KC, HT], bf16, tag="qT", name=f"qT_{h}")
                for mc in range(KC):
                    ps = psum.tile([P, HT], f32, tag="mm", name=f"ps_q_{h}_{mc}")
                    for ko in range(KC):
                        nc.tensor.matmul(ps, wq_b[:, ko, mc * P:(mc + 1) * P],
                                         x_b[:, ko, :],
                                         start=(ko == 0), stop=(ko == KC - 1))
                    nc.scalar.activation(qT[:, mc, :], ps, AF.Identity,
                                         bias=bq_t[:, mc:mc + 1])

                # expT [lc, hw] = exp(scale * kT.T q)
                eT = work.tile([P, KL, HT], bf16, tag="eT", name=f"eT_{h}")
                for ml in range(KL):
                    ps = psum.tile([P, HT], f32, tag="mm", name=f"ps_s_{h}_{ml}")
                    for mc in range(KC):
                        nc.tensor.matmul(ps, kT_b[:, mc, ml * P:(ml + 1) * P],
                                         qT[:, mc, :],
                                         start=(mc == 0), stop=(mc == KC - 1))
                    nc.scalar.activation(eT[:, ml, :], ps, AF.Exp, scale=SCALE)

                # softmax denominator: ones^T @ expT, then reciprocal, then
                # broadcast across partitions via K=1 matmul
                ps_sum = psum_s.tile([1, HT], f32, tag="sum", name=f"ps_sum_{h}")
                for ml in range(KL):
                    nc.tensor.matmul(ps_sum, ones_col, eT[:, ml, :],
                                     start=(ml == 0), stop=(ml == KL - 1))
                rec_f = small.tile([1, HT], f32, tag="rec_f", name=f"rec_f_{h}")
                nc.vector.reciprocal(rec_f, ps_sum)
                rec_b = small.tile([1, HT], bf16, tag="rec_b", name=f"rec_b_{h}")
                nc.vector.tensor_copy(out=rec_b, in_=rec_f)
                ps_bc = psum_bc.tile([P, HT], f32, tag="bc", name=f"ps_bc_{h}")
                nc.tensor.matmul(ps_bc, ones_row, rec_b, start=True, stop=True)
                rec_sb = work.tile([P, HT], bf16, tag="rec_sb", name=f"rec_sb_{h}")
                nc.vector.tensor_copy(out=rec_sb, in_=ps_bc)

                # outT [c, hw] = (v^T expT) * recip  (normalization folded in)
                oT = work.tile([P, KC, HT], bf16, tag="oT", name=f"oT_{h}")
                for mc in range(KC):
                    ps = psum.tile([P, HT], f32, tag="mm", name=f"ps_o_{h}_{mc}")
                    for ml in range(KL):
                        nc.tensor.matmul(ps, v_b[:, ml, mc * P:(mc + 1) * P],
                                         eT[:, ml, :],
                                         start=(ml == 0), stop=(ml == KL - 1))
                    nc.vector.tensor_mul(out=oT[:, mc, :], in0=ps, in1=rec_sb)

                # yT [c_out, hw] = Wo.T outT + bo + x
                y_sb = yout.tile([P, KC, HT], f32, tag="y", name=f"y_{h}")
                for mo in range(KC):
                    ps = psum.tile([P, HT], f32, tag="mm", name=f"ps_y_{h}_{mo}")
                    for mc in range(KC):
                        nc.tensor.matmul(ps, wo_b[:, mc, mo * P:(mo + 1) * P],
                                         oT[:, mc, :],
                                         start=(mc == 0), stop=(mc == KC - 1))
                    nc.scalar.activation(y_sb[:, mo, :], ps, AF.Identity,
                                         bias=bo_t[:, mo:mo + 1])
                    nc.vector.tensor_add(out=y_sb[:, mo, :], in0=y_sb[:, mo, :],
                                         in1=x_f[:, mo, :])
                nc.sync.dma_start(out=y_r[:, :, hs], in_=y_sb)

    nc.compile()
    return nc


def _get_compiled():
    if "nc" not in _cache:
        _cache["nc"] = _build_nc()
    return _cache["nc"]


def _make_in_maps(x, context, Wq, bq, Wk, bk, Wv, bv, Wo, bo):
    x = np.ascontiguousarray(np.asarray(x, dtype=np.float32))
    context = np.ascontiguousarray(np.asarray(context, dtype=np.float32))
    common = {
        "wq": np.ascontiguousarray(np.asarray(Wq, dtype=np.float32)),
        "wk": np.ascontiguousarray(np.asarray(Wk, dtype=np.float32)),
        "wv": np.ascontiguousarray(np.asarray(Wv, dtype=np.float32)),
        "wo": np.ascontiguousarray(np.asarray(Wo, dtype=np.float32)),
        "bq": np.ascontiguousarray(np.asarray(bq, dtype=np.float32)),
        "bk": np.ascontiguousarray(np.asarray(bk, dtype=np.float32)),
        "bv": np.ascontiguousarray(np.asarray(bv, dtype=np.float32)),
        "bo": np.ascontiguousarray(np.asarray(bo, dtype=np.float32)),
    }
    in_maps = []
    for b in range(B):
        m = dict(common)
        m["x"] = np.ascontiguousarray(x[b].reshape(C, HW))
        m["ctx"] = np.ascontiguousarray(context[b])
        in_maps.append(m)
    return in_maps


def _run(in_maps, trace=False):
    from concourse.bass_utils import run_bass_kernel_spmd
    nc = _get_compiled()
    return run_bass_kernel_spmd(nc, in_maps, core_ids=list(range(N_CORES)),
                                trace=trace)


def kernel(x, context, Wq, bq, Wk, bk, Wv, bv, Wo, bo):
    in_maps = _make_in_maps(x, context, Wq, bq, Wk, bk, Wv, bv, Wo, bo)
    res = _run(in_maps, trace=False)
    out = np.stack([res.results[b]["y"].reshape(C, HH, WW) for b in range(B)])
    return out.astype(np.float32)


# revision 20
# speedup vs baseline: 1.4523x; 1.4523x over previous
"""Cross-attention (single-head, residual) Bass/Tile kernel for Trainium2.

Problem: y = x + (softmax((x' Wq + bq)(ctx Wk + bk)^T / sqrt(C)) (ctx Wv + bv)) Wo + bo
  x: [B=8, C=512, H=64, W=64], context: [B=8, Lc=512, CTX=768]

Sharding: pure data-parallel over batch — one batch element per NeuronCore,
no collectives.

Dtype strategy: tensors DMA'd from HBM (x, weights, context) stay fp32 in
SBUF and are fed to the PE as float32r (full-rate at free-dim 512), so no
cast passes exist anywhere.  Engine-produced intermediates (q^T, k^T, v,
exp(sim^T)) are written as bf16 during their mandatory PSUM evictions.  All
accumulation is fp32 in PSUM; the residual add and output are fp32.

Layouts: every matmul is arranged contraction-major (out = lhsT.T @ rhs), so
x ([C, HW]) and all weights are used in their natural layouts; only
`context` needs an on-chip transpose (24 PE-transpose tiles).  Softmax runs
in the [lc, hw] layout: no max-subtraction (logits are O(1) by
construction), denominator via ones-vector matmul partition-reduction,
reciprocal via the fast custom-DVE op, broadcast across partitions via a
K=1 matmul, and the division is folded into the attn@V PSUM eviction.
"""

import numpy as np

B = 8
C = 512
CTX = 768
Lc = 512
HH = 64
WW = 64
HW = HH * WW          # 4096
N_CORES = 8
P = 128
HT = 512              # hw tile (free-dim) width
N_HT = HW // HT       # 8
KC = C // P           # 4
KX = CTX // P         # 6
KL = Lc // P          # 4
SCALE = float(C) ** -0.5

_cache = {}


def _build_nc():
    import concourse.mybir as mybir
    import concourse.bass as bass
    import concourse.tile as tile
    from concourse import bacc
    from concourse.masks import make_identity

    f32 = mybir.dt.float32
    f32r = mybir.dt.float32r
    bf16 = mybir.dt.bfloat16
    AF = mybir.ActivationFunctionType

    nc = bacc.Bacc("TRN2", target_bir_lowering=False, debug=False,
                   num_devices=N_CORES)

    x_d = nc.dram_tensor("x", [C, HW], f32r, kind="ExternalInput").ap()
    ctx_d = nc.dram_tensor("ctx", [Lc, CTX], f32, kind="ExternalInput").ap()
    wq_d = nc.dram_tensor("wq", [C, C], f32r, kind="ExternalInput").ap()
    wk_d = nc.dram_tensor("wk", [CTX, C], f32r, kind="ExternalInput").ap()
    wv_d = nc.dram_tensor("wv", [CTX, C], f32r, kind="ExternalInput").ap()
    wo_d = nc.dram_tensor("wo", [C, C], f32r, kind="ExternalInput").ap()
    bq_d = nc.dram_tensor("bq", [C], f32, kind="ExternalInput").ap()
    bk_d = nc.dram_tensor("bk", [C], f32, kind="ExternalInput").ap()
    bv_d = nc.dram_tensor("bv", [C], f32, kind="ExternalInput").ap()
    bo_d = nc.dram_tensor("bo", [C], f32, kind="ExternalInput").ap()
    y_d = nc.dram_tensor("y", [C, HW], f32, kind="ExternalOutput").ap()

    x_r = x_d.rearrange("(ko p) hw -> p ko hw", p=P)      # [128, 4, 4096]
    y_r = y_d.rearrange("(ko p) hw -> p ko hw", p=P)
    ctx_r = ctx_d.rearrange("(lo p) cx -> p lo cx", p=P)  # [128, 4, 768]
    wq_r = wq_d.rearrange("(ko p) c -> p ko c", p=P)      # [128, 4, 512]
    wk_r = wk_d.rearrange("(ko p) c -> p ko c", p=P)      # [128, 6, 512]
    wv_r = wv_d.rearrange("(ko p) c -> p ko c", p=P)
    wo_r = wo_d.rearrange("(ko p) c -> p ko c", p=P)

    def r(ap):  # feed fp32 SBUF data to the PE at full rate
        return ap.bitcast(mybir.dt.float32r)

    with tile.TileContext(nc) as tc:
        with (
            tc.tile_pool(name="const", bufs=1) as const,
            tc.tile_pool(name="xin", bufs=3) as xin,
            tc.tile_pool(name="work", bufs=2) as work,
            tc.tile_pool(name="yout", bufs=2) as yout,
            tc.tile_pool(name="small", bufs=3) as small,
            tc.tile_pool(name="psum", bufs=6, space="PSUM") as psum,
            tc.tile_pool(name="psum_s", bufs=1, space="PSUM") as psum_s,
            tc.tile_pool(name="psum_bc", bufs=1, space="PSUM") as psum_bc,
        ):
            # ---------------- phase A: weights / context prep ----------------
            wq_f = const.tile([P, KC, C], f32r, name="wq_f", tag="wq_f")
            nc.sync.dma_start(out=wq_f, in_=wq_r)

            # prefetch first x tiles before the bulkier context/weight DMAs
            x_tiles = {}
            for h in range(2):
                x_f = xin.tile([P, KC, HT], f32r, tag="x_f", name=f"x_f_{h}")
                nc.sync.dma_start(out=x_f, in_=x_r[:, :, h * HT:(h + 1) * HT])
                x_tiles[h] = x_f

            ctx_f = const.tile([P, KL, CTX], f32, name="ctx_f", tag="ctx_f")
            wk_f = const.tile([P, KX, C], f32r, name="wk_f", tag="wk_f")
            wv_f = const.tile([P, KX, C], f32r, name="wv_f", tag="wv_f")
            wo_f = const.tile([P, KC, C], f32r, name="wo_f", tag="wo_f")
            nc.sync.dma_start(out=ctx_f, in_=ctx_r)
            nc.sync.dma_start(out=wk_f, in_=wk_r)
            nc.sync.dma_start(out=wv_f, in_=wv_r)
            nc.sync.dma_start(out=wo_f, in_=wo_r)

            ident_f = const.tile([P, P], f32, name="ident_f", tag="ident")
            make_identity(nc, ident_f)
            ones_col = const.tile([P, 1], bf16, name="ones_col", tag="ones_c")
            nc.vector.memset(ones_col, 1.0)
            ones_row = const.tile([1, P], bf16, name="ones_row", tag="ones_r")
            nc.vector.memset(ones_row, 1.0)

            # biases: per-partition layout [128, KC]
            bq_t = const.tile([P, KC], f32, name="bq_t", tag="bq")
            bk_t = const.tile([P, KC], f32, name="bk_t", tag="bk")
            bo_t = const.tile([P, KC], f32, name="bo_t", tag="bo")
            with nc.allow_non_contiguous_dma(reason="tiny one-time bias loads"):
                nc.sync.dma_start(out=bq_t, in_=bq_d.rearrange("(ko p) -> p ko", p=P))
                nc.sync.dma_start(out=bk_t, in_=bk_d.rearrange("(ko p) -> p ko", p=P))
                nc.sync.dma_start(out=bo_t, in_=bo_d.rearrange("(ko p) -> p ko", p=P))
            # bv broadcast across partitions: [128, 512]
            bv_bc = const.tile([P, C], f32, name="bv_bc", tag="bv")
            bv_src = bass.AP(tensor=bv_d.tensor, offset=bv_d.offset,
                             ap=[[0, P]] + list(bv_d.ap))
            nc.sync.dma_start(out=bv_bc, in_=bv_src)

            # context transpose: ctxT [128(cx), KX, Lc] fp32
            ctxT_f = const.tile([P, KX, Lc], f32, name="ctxT_f", tag="ctxT")
            for lo in range(KL):
                for cx in range(KX):
                    ps_t = psum.tile([P, P], f32, tag="mm", name=f"ps_t_{lo}_{cx}")
                    nc.tensor.transpose(ps_t, ctx_f[:, lo, cx * P:(cx + 1) * P],
                                        ident_f)
                    if cx % 2 == 0:
                        nc.scalar.activation(r(ctxT_f[:, cx, lo * P:(lo + 1) * P]),
                                             ps_t, AF.Copy)
                    else:
                        nc.vector.tensor_copy(
                            out=r(ctxT_f[:, cx, lo * P:(lo + 1) * P]), in_=ps_t)

            # kT [128(c), KC, Lc] = (ctx Wk + bk)^T ; v [128(lc), KL, C] = ctx Wv + bv
            kT_b = const.tile([P, KC, Lc], bf16, name="kT_b", tag="kT")
            v_b = const.tile([P, KL, C], bf16, name="v_b", tag="v")
            for mc in range(KC):
                ps = psum.tile([P, Lc], f32, tag="mm", name=f"ps_k_{mc}")
                for cx in range(KX):
                    nc.tensor.matmul(ps, wk_f[:, cx, mc * P:(mc + 1) * P],
                                     r(ctxT_f[:, cx, :]),
                                     start=(cx == 0), stop=(cx == KX - 1))
                nc.scalar.activation(kT_b[:, mc, :], ps, AF.Identity,
                                     bias=bk_t[:, mc:mc + 1])
            for ml in range(KL):
                ps = psum.tile([P, C], f32, tag="mm", name=f"ps_v_{ml}")
                for cx in range(KX):
                    nc.tensor.matmul(ps, r(ctxT_f[:, cx, ml * P:(ml + 1) * P]),
                                     wv_f[:, cx, :],
                                     start=(cx == 0), stop=(cx == KX - 1))
                nc.vector.tensor_add(out=v_b[:, ml, :], in0=ps, in1=bv_bc)

            # ---------------- phase B: stream over hw tiles ----------------
            for h in range(N_HT):
                hs = slice(h * HT, (h + 1) * HT)

                if h in x_tiles:
                    x_f = x_tiles[h]
                else:
                    x_f = xin.tile([P, KC, HT], f32r, tag="x_f", name=f"x_f_{h}")
                    nc.sync.dma_start(out=x_f, in_=x_r[:, :, hs])

                # qT [c, hw] (bf16 via eviction)
                qT = work.tile([P, KC, HT], bf16, tag="qT", name=f"qT_{h}")
                for mc in range(KC):
                    ps = psum.tile([P, HT], f32, tag="mm", name=f"ps_q_{h}_{mc}")
                    for ko in range(KC):
                        nc.tensor.matmul(ps, wq_f[:, ko, mc * P:(mc + 1) * P],
                                         x_f[:, ko, :],
                                         start=(ko == 0), stop=(ko == KC - 1))
                    nc.scalar.activation(qT[:, mc, :], ps, AF.Identity,
                                         bias=bq_t[:, mc:mc + 1])

                # expT [lc, hw] = exp(scale * kT.T q)
                eT = work.tile([P, KL, HT], bf16, tag="eT", name=f"eT_{h}")
                for ml in range(KL):
                    ps = psum.tile([P, HT], f32, tag="mm", name=f"ps_s_{h}_{ml}")
                    for mc in range(KC):
                        nc.tensor.matmul(ps, kT_b[:, mc, ml * P:(ml + 1) * P],
                                         qT[:, mc, :],
                                         start=(mc == 0), stop=(mc == KC - 1))
                    nc.scalar.activation(eT[:, ml, :], ps, AF.Exp, scale=SCALE)

                # softmax denominator: ones^T @ expT -> broadcast -> 1/sum
                ps_sum = psum_s.tile([1, HT], f32, tag="sum", name=f"ps_sum_{h}")
                for ml in range(KL):
                    nc.tensor.matmul(ps_sum, ones_col, eT[:, ml, :],
                                     start=(ml == 0), stop=(ml == KL - 1))
                sum_sb = small.tile([1, HT], bf16, tag="sum_sb", name=f"sum_sb_{h}")
                nc.scalar.activation(sum_sb, ps_sum, AF.Copy)
                ps_bc = psum_bc.tile([P, HT], f32, tag="bc", name=f"ps_bc_{h}")
                nc.tensor.matmul(ps_bc, ones_row, sum_sb, start=True, stop=True)
                rec_sb = work.tile([P, HT], f32, tag="rec_sb", name=f"rec_sb_{h}")
                nc.vector.reciprocal_approx_fast(out=rec_sb, in_=ps_bc)

                # outT [c, hw] = (v^T expT) * recip  (normalization folded in)
                oT = work.tile([P, KC, HT], f32, tag="oT", name=f"oT_{h}")
                for mc in range(KC):
                    ps = psum.tile([P, HT], f32, tag="mm", name=f"ps_o_{h}_{mc}")
                    for ml in range(KL):
                        nc.tensor.matmul(ps, v_b[:, ml, mc * P:(mc + 1) * P],
                                         eT[:, ml, :],
                                         start=(ml == 0), stop=(ml == KL - 1))
                    nc.vector.tensor_mul(out=r(oT[:, mc, :]), in0=ps, in1=rec_sb)

                # yT [c_out, hw] = Wo.T outT + bo + x
                y_sb = yout.tile([P, KC, HT], f32, tag="y", name=f"y_{h}")
                for mo in range(KC):
                    ps = psum.tile([P, HT], f32, tag="mm", name=f"ps_y_{h}_{mo}")
                    for mc in range(KC):
                        nc.tensor.matmul(ps, wo_f[:, mc, mo * P:(mo + 1) * P],
                                         r(oT[:, mc, :]),
                                         start=(mc == 0), stop=(mc == KC - 1))
                    nc.scalar.activation(y_sb[:, mo, :], ps, AF.Identity,
                                         bias=bo_t[:, mo:mo + 1])
                    nc.gpsimd.tensor_add(out=y_sb[:, mo, :], in0=y_sb[:, mo, :],
                                         in1=x_f[:, mo, :].bitcast(f32))
                nc.sync.dma_start(out=y_r[:, :, hs], in_=y_sb)

    nc.compile()
    return nc


def _get_compiled():
    if "nc" not in _cache:
        _cache["nc"] = _build_nc()
    return _cache["nc"]


def _make_in_maps(x, context, Wq, bq, Wk, bk, Wv, bv, Wo, bo):
    x = np.ascontiguousarray(np.asarray(x, dtype=np.float32))
    context = np.ascontiguousarray(np.asarray(context, dtype=np.float32))
    common = {
        "wq": np.ascontiguousarray(np.asarray(Wq, dtype=np.float32)),
        "wk": np.ascontiguousarray(np.asarray(Wk, dtype=np.float32)),
        "wv": np.ascontiguousarray(np.asarray(Wv, dtype=np.float32)),
        "wo": np.ascontiguousarray(np.asarray(Wo, dtype=np.float32)),
        "bq": np.ascontiguousarray(np.asarray(bq, dtype=np.float32)),
        "bk": np.ascontiguousarray(np.asarray(bk, dtype=np.float32)),
        "bv": np.ascontiguousarray(np.asarray(bv, dtype=np.float32)),
        "bo": np.ascontiguousarray(np.asarray(bo, dtype=np.float32)),
    }
    in_maps = []
    for b in range(B):
        m = dict(common)
        m["x"] = np.ascontiguousarray(x[b].reshape(C, HW))
        m["ctx"] = np.ascontiguousarray(context[b])
        in_maps.append(m)
    return in_maps


def _run(in_maps, trace=False):
    from concourse.bass_utils import run_bass_kernel_spmd
    nc = _get_compiled()
    return run_bass_kernel_spmd(nc, in_maps, core_ids=list(range(N_CORES)),
                                trace=trace)


def kernel(x, context, Wq, bq, Wk, bk, Wv, bv, Wo, bo):
    in_maps = _make_in_maps(x, context, Wq, bq, Wk, bk, Wv, bv, Wo, bo)
    res = _run(in_maps, trace=False)
    out = np.stack([res.results[b]["y"].reshape(C, HH, WW) for b in range(B)])
    return out.astype(np.float32)


# revision 22
# speedup vs baseline: 1.5182x; 1.0453x over previous
"""Cross-attention (single-head, residual) Bass/Tile kernel for Trainium2.

Problem: y = x + (softmax((x' Wq + bq)(ctx Wk + bk)^T / sqrt(C)) (ctx Wv + bv)) Wo + bo
  x: [B=8, C=512, H=64, W=64], context: [B=8, Lc=512, CTX=768]

Sharding: pure data-parallel over batch — one batch element per NeuronCore,
no collectives.

Dtype strategy: tensors DMA'd from HBM (x, weights, context) stay fp32 in
SBUF and are fed to the PE as float32r (full-rate at free-dim 512), so no
cast passes exist anywhere.  Engine-produced intermediates (q^T, k^T, v,
exp(sim^T)) are written as bf16 during their mandatory PSUM evictions.  All
accumulation is fp32 in PSUM; the residual add and output are fp32.

Layouts: every matmul is arranged contraction-major (out = lhsT.T @ rhs), so
x ([C, HW]) and all weights are used in their natural layouts; only
`context` needs an on-chip transpose (24 PE-transpose tiles).  Softmax runs
in the [lc, hw] layout: no max-subtraction (logits are O(1) by
construction), denominator via ones-vector matmul partition-reduction,
reciprocal via the fast custom-DVE op, broadcast across partitions via a
K=1 matmul, and the division is folded into the attn@V PSUM eviction.
"""

import numpy as np

B = 8
C = 512
CTX = 768
Lc = 512
HH = 64
WW = 64
HW = HH * WW          # 4096
N_CORES = 8
P = 128
HT = 512              # hw tile (free-dim) width
N_HT = HW // HT       # 8
KC = C // P           # 4
KX = CTX // P         # 6
KL = Lc // P          # 4
SCALE = float(C) ** -0.5

_cache = {}


def _build_nc():
    import concourse.mybir as mybir
    import concourse.bass as bass
    import concourse.tile as tile
    from concourse import bacc
    from concourse.masks import make_identity

    f32 = mybir.dt.float32
    f32r = mybir.dt.float32r
    bf16 = mybir.dt.bfloat16
    AF = mybir.ActivationFunctionType

    nc = bacc.Bacc("TRN2", target_bir_lowering=False, debug=False,
                   num_devices=N_CORES)

    x_d = nc.dram_tensor("x", [C, HW], f32r, kind="ExternalInput").ap()
    ctx_d = nc.dram_tensor("ctx", [Lc, CTX], f32, kind="ExternalInput").ap()
    wq_d = nc.dram_tensor("wq", [C, C], f32r, kind="ExternalInput").ap()
    wk_d = nc.dram_tensor("wk", [CTX, C], f32r, kind="ExternalInput").ap()
    wv_d = nc.dram_tensor("wv", [CTX, C], f32r, kind="ExternalInput").ap()
    wo_d = nc.dram_tensor("wo", [C, C], f32r, kind="ExternalInput").ap()
    bq_d = nc.dram_tensor("bq", [C], f32, kind="ExternalInput").ap()
    bk_d = nc.dram_tensor("bk", [C], f32, kind="ExternalInput").ap()
    bv_d = nc.dram_tensor("bv", [C], f32, kind="ExternalInput").ap()
    bo_d = nc.dram_tensor("bo", [C], f32, kind="ExternalInput").ap()
    y_d = nc.dram_tensor("y", [C, HW], f32, kind="ExternalOutput").ap()

    x_r = x_d.rearrange("(ko p) hw -> p ko hw", p=P)      # [128, 4, 4096]
    y_r = y_d.rearrange("(ko p) hw -> p ko hw", p=P)
    ctx_r = ctx_d.rearrange("(lo p) cx -> p lo cx", p=P)  # [128, 4, 768]
    wq_r = wq_d.rearrange("(ko p) c -> p ko c", p=P)      # [128, 4, 512]
    wk_r = wk_d.rearrange("(ko p) c -> p ko c", p=P)      # [128, 6, 512]
    wv_r = wv_d.rearrange("(ko p) c -> p ko c", p=P)
    wo_r = wo_d.rearrange("(ko p) c -> p ko c", p=P)

    def r(ap):  # feed fp32 SBUF data to the PE at full rate
        return ap.bitcast(mybir.dt.float32r)

    with tile.TileContext(nc) as tc:
        with (
            tc.tile_pool(name="const", bufs=1) as const,
            tc.tile_pool(name="xin", bufs=3) as xin,
            tc.tile_pool(name="work", bufs=2) as work,
            tc.tile_pool(name="yout", bufs=2) as yout,
            tc.tile_pool(name="small", bufs=3) as small,
            tc.tile_pool(name="psum", bufs=6, space="PSUM") as psum,
            tc.tile_pool(name="psum_s", bufs=1, space="PSUM") as psum_s,
            tc.tile_pool(name="psum_bc", bufs=1, space="PSUM") as psum_bc,
        ):
            # ---------------- phase A: weights / context prep ----------------
            wq_f = const.tile([P, KC, C], f32r, name="wq_f", tag="wq_f")
            nc.sync.dma_start(out=wq_f, in_=wq_r)

            # prefetch first x tiles before the bulkier context/weight DMAs
            x_tiles = {}
            for h in range(2):
                x_f = xin.tile([P, KC, HT], f32r, tag="x_f", name=f"x_f_{h}")
                nc.sync.dma_start(out=x_f, in_=x_r[:, :, h * HT:(h + 1) * HT])
                x_tiles[h] = x_f

            ctx_f = const.tile([P, KL, CTX], f32, name="ctx_f", tag="ctx_f")
            wk_f = const.tile([P, KX, C], f32r, name="wk_f", tag="wk_f")
            wv_f = const.tile([P, KX, C], f32r, name="wv_f", tag="wv_f")
            wo_f = const.tile([P, KC, C], f32r, name="wo_f", tag="wo_f")
            for lo in range(KL):  # chunked so transposes start early
                nc.sync.dma_start(out=ctx_f[:, lo, :], in_=ctx_r[:, lo, :])
            nc.sync.dma_start(out=wk_f, in_=wk_r)
            nc.sync.dma_start(out=wv_f, in_=wv_r)
            nc.sync.dma_start(out=wo_f, in_=wo_r)

            ident_f = const.tile([P, P], f32, name="ident_f", tag="ident")
            make_identity(nc, ident_f)
            ones_col = const.tile([P, 1], bf16, name="ones_col", tag="ones_c")
            nc.vector.memset(ones_col, 1.0)
            ones_row = const.tile([1, P], bf16, name="ones_row", tag="ones_r")
            nc.vector.memset(ones_row, 1.0)

            # biases: per-partition layout [128, KC]; tiny scattered DMAs go
            # on the gpsimd queue to keep the sync queue for bulk transfers
            bq_t = const.tile([P, KC], f32, name="bq_t", tag="bq")
            bk_t = const.tile([P, KC], f32, name="bk_t", tag="bk")
            bo_t = const.tile([P, KC], f32, name="bo_t", tag="bo")
            with nc.allow_non_contiguous_dma(reason="tiny one-time bias loads"):
                nc.gpsimd.dma_start(out=bq_t, in_=bq_d.rearrange("(ko p) -> p ko", p=P))
                nc.gpsimd.dma_start(out=bk_t, in_=bk_d.rearrange("(ko p) -> p ko", p=P))
                nc.gpsimd.dma_start(out=bo_t, in_=bo_d.rearrange("(ko p) -> p ko", p=P))
            # bv broadcast across partitions: [128, 512]
            bv_bc = const.tile([P, C], f32, name="bv_bc", tag="bv")
            bv_src = bass.AP(tensor=bv_d.tensor, offset=bv_d.offset,
                             ap=[[0, P]] + list(bv_d.ap))
            nc.gpsimd.dma_start(out=bv_bc, in_=bv_src)

            ctxT_f = const.tile([P, KX, Lc], f32, name="ctxT_f", tag="ctxT")
            kT_b = const.tile([P, KC, Lc], bf16, name="kT_b", tag="kT")
            v_b = const.tile([P, KL, C], bf16, name="v_b", tag="v")

            def emit_context_prep():
                # context transpose: ctxT [128(cx), KX, Lc] fp32
                for lo in range(KL):
                    for cx in range(KX):
                        ps_t = psum.tile([P, P], f32, tag="mm",
                                         name=f"ps_t_{lo}_{cx}")
                        nc.tensor.transpose(ps_t, ctx_f[:, lo, cx * P:(cx + 1) * P],
                                            ident_f)
                        if cx % 2 == 0:
                            nc.scalar.activation(
                                r(ctxT_f[:, cx, lo * P:(lo + 1) * P]), ps_t, AF.Copy)
                        else:
                            nc.vector.tensor_copy(
                                out=r(ctxT_f[:, cx, lo * P:(lo + 1) * P]), in_=ps_t)
                # kT = (ctx Wk + bk)^T ; v = ctx Wv + bv
                for mc in range(KC):
                    ps = psum.tile([P, Lc], f32, tag="mm", name=f"ps_k_{mc}")
                    for cx in range(KX):
                        nc.tensor.matmul(ps, wk_f[:, cx, mc * P:(mc + 1) * P],
                                         r(ctxT_f[:, cx, :]),
                                         start=(cx == 0), stop=(cx == KX - 1))
                    nc.scalar.activation(kT_b[:, mc, :], ps, AF.Identity,
                                         bias=bk_t[:, mc:mc + 1])
                for ml in range(KL):
                    ps = psum.tile([P, C], f32, tag="mm", name=f"ps_v_{ml}")
                    for cx in range(KX):
                        nc.tensor.matmul(ps, r(ctxT_f[:, cx, ml * P:(ml + 1) * P]),
                                         wv_f[:, cx, :],
                                         start=(cx == 0), stop=(cx == KX - 1))
                    nc.vector.tensor_add(out=v_b[:, ml, :], in0=ps, in1=bv_bc)

            # ---------------- phase B: stream over hw tiles ----------------
            for h in range(N_HT):
                hs = slice(h * HT, (h + 1) * HT)

                if h in x_tiles:
                    x_f = x_tiles[h]
                else:
                    x_f = xin.tile([P, KC, HT], f32r, tag="x_f", name=f"x_f_{h}")
                    nc.sync.dma_start(out=x_f, in_=x_r[:, :, hs])

                # qT [c, hw] (bf16 via eviction)
                qT = work.tile([P, KC, HT], bf16, tag="qT", name=f"qT_{h}")
                for mc in range(KC):
                    ps = psum.tile([P, HT], f32, tag="mm", name=f"ps_q_{h}_{mc}")
                    for ko in range(KC):
                        nc.tensor.matmul(ps, wq_f[:, ko, mc * P:(mc + 1) * P],
                                         x_f[:, ko, :],
                                         start=(ko == 0), stop=(ko == KC - 1))
                    nc.scalar.activation(qT[:, mc, :], ps, AF.Identity,
                                         bias=bq_t[:, mc:mc + 1])

                if h == 0:
                    # context prep lands on the PE right after the first qT
                    # block, by which time the ctx/wk/wv DMAs have arrived
                    emit_context_prep()

                # expT [lc, hw] = exp(scale * kT.T q)
                eT = work.tile([P, KL, HT], bf16, tag="eT", name=f"eT_{h}")
                for ml in range(KL):
                    ps = psum.tile([P, HT], f32, tag="mm", name=f"ps_s_{h}_{ml}")
                    for mc in range(KC):
                        nc.tensor.matmul(ps, kT_b[:, mc, ml * P:(ml + 1) * P],
                                         qT[:, mc, :],
                                         start=(mc == 0), stop=(mc == KC - 1))
                    nc.scalar.activation(eT[:, ml, :], ps, AF.Exp, scale=SCALE)

                # softmax denominator: ones^T @ expT -> broadcast -> 1/sum
                ps_sum = psum_s.tile([1, HT], f32, tag="sum", name=f"ps_sum_{h}")
                for ml in range(KL):
                    nc.tensor.matmul(ps_sum, ones_col, eT[:, ml, :],
                                     start=(ml == 0), stop=(ml == KL - 1))
                sum_sb = small.tile([1, HT], bf16, tag="sum_sb", name=f"sum_sb_{h}")
                nc.scalar.activation(sum_sb, ps_sum, AF.Copy)
                ps_bc = psum_bc.tile([P, HT], f32, tag="bc", name=f"ps_bc_{h}")
                nc.tensor.matmul(ps_bc, ones_row, sum_sb, start=True, stop=True)
                rec_sb = work.tile([P, HT], f32, tag="rec_sb", name=f"rec_sb_{h}")
                nc.vector.reciprocal_approx_fast(out=rec_sb, in_=ps_bc)

                # outT [c, hw] = (v^T expT) * recip  (normalization folded in)
                oT = work.tile([P, KC, HT], f32, tag="oT", name=f"oT_{h}")
                for mc in range(KC):
                    ps = psum.tile([P, HT], f32, tag="mm", name=f"ps_o_{h}_{mc}")
                    for ml in range(KL):
                        nc.tensor.matmul(ps, v_b[:, ml, mc * P:(mc + 1) * P],
                                         eT[:, ml, :],
                                         start=(ml == 0), stop=(ml == KL - 1))
                    nc.vector.tensor_mul(out=r(oT[:, mc, :]), in0=ps, in1=rec_sb)

                # yT [c_out, hw] = Wo.T outT + bo + x
                y_sb = yout.tile([P, KC, HT], f32, tag="y", name=f"y_{h}")
                for mo in range(KC):
                    ps = psum.tile([P, HT], f32, tag="mm", name=f"ps_y_{h}_{mo}")
                    for mc in range(KC):
                        nc.tensor.matmul(ps, wo_f[:, mc, mo * P:(mo + 1) * P],
                                         r(oT[:, mc, :]),
                                         start=(mc == 0), stop=(mc == KC - 1))
                    nc.scalar.activation(y_sb[:, mo, :], ps, AF.Identity,
                                         bias=bo_t[:, mo:mo + 1])
                    nc.gpsimd.tensor_add(out=y_sb[:, mo, :], in0=y_sb[:, mo, :],
                                         in1=x_f[:, mo, :].bitcast(f32))
                    nc.sync.dma_start(out=y_r[:, mo, hs], in_=y_sb[:, mo, :])

    nc.compile()
    return nc


def _get_compiled():
    if "nc" not in _cache:
        _cache["nc"] = _build_nc()
    return _cache["nc"]


def _make_in_maps(x, context, Wq, bq, Wk, bk, Wv, bv, Wo, bo):
    x = np.ascontiguousarray(np.asarray(x, dtype=np.float32))
    context = np.ascontiguousarray(np.asarray(context, dtype=np.float32))
    common = {
        "wq": np.ascontiguousarray(np.asarray(Wq, dtype=np.float32)),
        "wk": np.ascontiguousarray(np.asarray(Wk, dtype=np.float32)),
        "wv": np.ascontiguousarray(np.asarray(Wv, dtype=np.float32)),
        "wo": np.ascontiguousarray(np.asarray(Wo, dtype=np.float32)),
        "bq": np.ascontiguousarray(np.asarray(bq, dtype=np.float32)),
        "bk": np.ascontiguousarray(np.asarray(bk, dtype=np.float32)),
        "bv": np.ascontiguousarray(np.asarray(bv, dtype=np.float32)),
        "bo": np.ascontiguousarray(np.asarray(bo, dtype=np.float32)),
    }
    in_maps = []
    for b in range(B):
        m = dict(common)
        m["x"] = np.ascontiguousarray(x[b].reshape(C, HW))
        m["ctx"] = np.ascontiguousarray(context[b])
        in_maps.append(m)
    return in_maps


def _run(in_maps, trace=False):
    from concourse.bass_utils import run_bass_kernel_spmd
    nc = _get_compiled()
    return run_bass_kernel_spmd(nc, in_maps, core_ids=list(range(N_CORES)),
                                trace=trace)


def kernel(x, context, Wq, bq, Wk, bk, Wv, bv, Wo, bo):
    in_maps = _make_in_maps(x, context, Wq, bq, Wk, bk, Wv, bv, Wo, bo)
    res = _run(in_maps, trace=False)
    out = np.stack([res.results[b]["y"].reshape(C, HH, WW) for b in range(B)])
    return out.astype(np.float32)
